# revision 40
# baseline (speedup 1.0000x reference)
"""Entropy-regularized attention (standard MHA fwd) on 8 trn2 cores.

Sharding: core c -> batch b=c//4, head-group g=c%4 (4 of 16 heads).
Each core computes q/k/v for its 256-wide head-group slice, transposed-
layout attention (scores^T = K^T-stationary matmuls, exp on ACT, AV with
v-stationary producing avT), then a row-split Wo partial product.
Host sums the 4 partials per batch and adds bo (the "all-reduce").

ACT-paced software pipeline with row-tiled score pairs. Blocks iterate
(q-block of 512, head-pair); each round computes BOTH heads' scores in
one [128,1024] PSUM tile via two concurrent row-tiled matmuls (heads sit
at array rows 0-63 / 64-127; contraction is hd=64), one exp ACTIVATE
covers both, then two AV matmuls accumulate per-head [65,512] banks
(65th v column = ones = softmax denominator). Projections / output
projection run as <=~430ns filler generators pumped between rounds,
with emission-order guards (Tile tracks deps in emission order) and the
output projections deferred to the late blocks where the filler queue
has drained. bf16 matmul inputs, fp16 output partials, DVE
reciprocal_approx_fast + GPSIMD partition_broadcast for the softmax
normalization, ACT-table warmup exp during the DMA head.

Measured: 216-220us vs 325-342us for the v1 baseline (~1.5x), ACT exp
stream 142.6us busy, PE ~179us busy; rel err 2.9e-3 (gate 2e-2).
"""

import sys

for _p in ("/opt/trn_rl_repo", "/root/.axon_site/_ro/trn_rl_repo"):
    if _p not in sys.path:
        sys.path.insert(0, _p)

import numpy as np

import concourse.bass as bass
import concourse.mybir as mybir
import concourse.tile as tile
from concourse import bacc

P = 128
S = 2048  # sequence length
D = 1024  # hidden
DG = 256  # per-core head-group width (4 heads x 64)
HD = 64
NHL = 4  # heads per core
KT_D = D // P  # 8 contraction tiles for projections
ST = S // P  # 16 sequence tiles
QB = 512  # q-block size
NQB = S // QB  # 4 q-blocks

F32 = mybir.dt.float32
F32R = mybir.dt.float32r
F16 = mybir.dt.float16
BF16 = mybir.dt.bfloat16


def build_nc():
    nc = bacc.Bacc(None, target_bir_lowering=False)

    xT = nc.dram_tensor("xT", [D, S], BF16, kind="ExternalInput")
    wq = nc.dram_tensor("wq", [D, DG], BF16, kind="ExternalInput")
    wk = nc.dram_tensor("wk", [D, DG], BF16, kind="ExternalInput")
    wv = nc.dram_tensor("wv", [D, DG], BF16, kind="ExternalInput")
    wo = nc.dram_tensor("wo", [DG, D], BF16, kind="ExternalInput")
    bq = nc.dram_tensor("bq", [P, 2], F32, kind="ExternalInput")
    bk = nc.dram_tensor("bk", [P, 2], F32, kind="ExternalInput")
    bv = nc.dram_tensor("bv", [1, DG], BF16, kind="ExternalInput")
    out = nc.dram_tensor("out", [S, D], F16, kind="ExternalOutput")

    with tile.TileContext(nc) as tc:
        _body(tc, nc, xT, wq, wk, wv, wo, bq, bk, bv, out)

    nc.compile()
    return nc


def _body(tc, nc, xT, wq, wk, wv, wo, bq, bk, bv, out):
    from contextlib import ExitStack

    with ExitStack() as ctx:
        ctx.enter_context(
            nc.allow_low_precision(
                reason="bf16 matmul inputs; accumulation is fp32 PSUM"
            )
        )
        persist = ctx.enter_context(tc.tile_pool(name="persist", bufs=1))
        expool = ctx.enter_context(tc.tile_pool(name="expool", bufs=3))
        npool = ctx.enter_context(tc.tile_pool(name="npool", bufs=4))
        opool = ctx.enter_context(tc.tile_pool(name="opool", bufs=2))
        # PSUM budget (8 banks): sc 2x[128,1024]=4, avA+avB 2x[128,512]=2,
        # fill 2x[128,512]=2.
        ps_sc = ctx.enter_context(tc.tile_pool(name="ps_sc", bufs=2, space="PSUM"))
        ps_av = ctx.enter_context(tc.tile_pool(name="ps_av", bufs=1, space="PSUM"))
        ps_fl = ctx.enter_context(tc.tile_pool(name="ps_fl", bufs=2, space="PSUM"))

        qT_sb = persist.tile([P, 2, S], BF16)
        kT_sb = persist.tile([P, 2, S], BF16)
        v_sb = persist.tile([P, ST, NHL * 65], BF16)  # 65-striped: col 64 = ones
        # avT[qb]: normalized attention output, [dg 128, kt2, 512 q] bf16
        avT = [
            persist.tile([P, 2, QB], BF16, tag=f"avT{qb}", name=f"avT{qb}")
            for qb in range(NQB)
        ]
        wo_sb = persist.tile([P, 2, D], BF16)
        ones_bf = persist.tile([1, P], BF16)
        xT_sb = persist.tile([P, KT_D, S], BF16)
        wq_sb = persist.tile([P, KT_D, DG], BF16, tag="wq")
        wk_sb = persist.tile([P, KT_D, DG], BF16, tag="wk")
        wv_sb = persist.tile([P, KT_D, DG], BF16, tag="wv")
        bq_sb = persist.tile([P, 2], F32, tag="bq")
        bk_sb = persist.tile([P, 2], F32, tag="bk")
        bv_sb = persist.tile([1, DG], BF16, tag="bv")

        # DMA order matters: wk/wq + xT q0 gate the first exp (K(0,0) and
        # Q(0,qb0) only need seq cols 0-511); wv gates V; wo only phase D.
        nc.sync.dma_start(bq_sb[:], bq[:])
        nc.sync.dma_start(bk_sb[:], bk[:])
        # wk first, then the xq0 chunks, THEN wq: the K chain starts on
        # wk + xq0[0] without waiting behind wq's megabyte; Q follows.
        nc.sync.dma_start(wk_sb[:], wk.rearrange("(kt p) n -> p kt n", p=P))
        xTr = xT.rearrange("(kt p) s -> p kt s", p=P)
        for kt in range(KT_D):
            nc.sync.dma_start(xT_sb[:, kt, 0:512], xTr[:, kt, 0:512])
        nc.sync.dma_start(wq_sb[:], wq.rearrange("(kt p) n -> p kt n", p=P))
        nc.sync.dma_start(bv_sb[:], bv[:])
        nc.sync.dma_start(wv_sb[:], wv.rearrange("(kt p) n -> p kt n", p=P))
        for kt in range(KT_D):
            nc.sync.dma_start(xT_sb[:, kt, 512:1024], xTr[:, kt, 512:1024])
        for sq in (2, 3):
            sl = slice(sq * 512, (sq + 1) * 512)
            nc.sync.dma_start(xT_sb[:, :, sl], xTr[:, :, sl])
        nc.sync.dma_start(wo_sb[:], wo.rearrange("(kt p) n -> p kt n", p=P))

        # constants: bf16/f32r ones rows and the ones stripe in v (softmax
        # denominator column)
        ones_f32 = persist.tile([P, P], F32)
        nc.vector.memset(ones_f32[:], 1.0)
        nc.vector.tensor_copy(ones_bf[:], ones_f32[0:1, :])
        nc.vector.tensor_copy(
            v_sb.rearrange("p st (h w) -> p st h w", w=65)[:, :, :, 64],
            ones_f32[:, 0:64].rearrange("p (st h) -> p st h", h=4),
        )
        # dummy exp during the DMA window so the ~2.7us ACT table load is
        # off the first real exp's critical path
        warm = persist.tile([1, 16], F32, tag="warm")
        nc.scalar.activation(
            warm[:], ones_f32[0:1, 0:16], mybir.ActivationFunctionType.Exp
        )

        # ---- filler generators (emitted in <=~430ns PE pieces) ----
        # `done` tracks which producer groups have been fully EMITTED.
        # Tile tracks deps in emission order, so a consumer emitted before
        # its producer sees a stale tile (WAR instead of RAW) — guards
        # below pump the queue until the producer is out.
        done = set()

        def gen_qk(kind, wsb, bsb, dest, mt, nq):
            # one [128, 512] slice of qT/kT: 8-MM K-chain + bias eviction
            ps = ps_fl.tile([P, 512], F32, tag="fill", name="qkps")
            for kt in range(KT_D):
                nc.tensor.matmul(
                    ps[:],
                    wsb[:, kt, mt * P : (mt + 1) * P],
                    xT_sb[:, kt, nq * 512 : (nq + 1) * 512],
                    start=(kt == 0),
                    stop=(kt == KT_D - 1),
                )
                if kt in (1, 3, 5):
                    yield
            nc.vector.tensor_scalar_add(
                dest[:, mt, nq * 512 : (nq + 1) * 512],
                ps[:],
                bsb[:, mt : mt + 1],
            )
            done.add((kind, mt, nq))
            yield

        def gen_v(st):
            ps = ps_fl.tile([P, 512], F32, tag="fill", name="vps")
            for kt in range(KT_D):
                nc.tensor.matmul(
                    ps[:, 0:DG],
                    xT_sb[:, kt, st * P : (st + 1) * P],
                    wv_sb[:, kt, :],
                    start=(kt == 0),
                    stop=False,
                )
                if kt in (3, 6):
                    yield
            nc.tensor.matmul(
                ps[:, 0:DG],
                ones_bf[0:1, 0:P],
                bv_sb[0:1, :],
                start=False,
                stop=True,
            )
            nc.vector.tensor_copy(
                v_sb.rearrange("p st (h w) -> p st h w", w=65)[:, st, :, 0:64],
                ps[:, 0:DG].rearrange("p (h w) -> p h w", w=64),
            )
            done.add(("v", st))
            yield

        def gen_norm(qb, h, ue, rinv):
            # avT[qb][head] = ue[0:64] * rinv: partition-broadcast of the
            # reciprocal row on (idle) GPSIMD, multiply on DVE — no PE work
            mt, po = h // 2, (h % 2) * 64
            rb = npool.tile([64, 512], F32, tag="rb")
            nc.gpsimd.partition_broadcast(rb[:], rinv[0:1, :], channels=64)
            nc.vector.tensor_mul(
                out=avT[qb][po : po + 64, mt, :],
                in0=ue[0:64, :],
                in1=rb[:],
            )
            done.add(("n", qb, h))
            yield

        def gen_phaseD(qb, sti):
            st = qb * (QB // P) + sti
            ot = opool.tile([P, D], F16, tag="ot")
            for nd in range(2):
                pp = ps_fl.tile([P, 512], F32, tag="fill", name="pp")
                for kt2 in range(2):
                    nc.tensor.matmul(
                        pp[:],
                        avT[qb][:, kt2, sti * P : (sti + 1) * P],
                        wo_sb[:, kt2, nd * 512 : (nd + 1) * 512],
                        start=(kt2 == 0),
                        stop=(kt2 == 1),
                    )
                nc.vector.tensor_copy(ot[:, nd * 512 : (nd + 1) * 512], pp[:])
                yield
            nc.sync.dma_start(out[st * P : (st + 1) * P, :], ot[:])
            yield

        def phaseD_tail(qb, sti):
            # tail variant: sc-pool [128,1024] as two pp banks, eviction
            # casts split DVE/ACT (both idle by now)
            st = qb * (QB // P) + sti
            ot = opool.tile([P, D], F16, tag="ot")
            pp = ps_sc.tile([P, 1024], F32, tag="sc", name="ppt")
            for nd in range(2):
                for kt2 in range(2):
                    nc.tensor.matmul(
                        pp[:, nd * 512 : (nd + 1) * 512],
                        avT[qb][:, kt2, sti * P : (sti + 1) * P],
                        wo_sb[:, kt2, nd * 512 : (nd + 1) * 512],
                        start=(kt2 == 0),
                        stop=(kt2 == 1),
                    )
            nc.vector.tensor_copy(ot[:, 0:512], pp[:, 0:512])
            nc.scalar.copy(ot[:, 512:1024], pp[:, 512:1024])
            nc.sync.dma_start(out[st * P : (st + 1) * P, :], ot[:])

        # ---- filler queue ----
        from collections import deque

        fill_q = deque()
        cur_gen = [None]

        def pump(n):
            for _ in range(n):
                while True:
                    if cur_gen[0] is None:
                        if not fill_q:
                            return
                        cur_gen[0] = fill_q.popleft()
                    try:
                        next(cur_gen[0])
                        break
                    except StopIteration:
                        cur_gen[0] = None

        def pump_until(key):
            while key not in done:
                assert cur_gen[0] is not None or fill_q, f"missing {key}"
                pump(1)

        # pre-attention (emitted directly, highest priority): the minimal
        # set gating exp(qb0,mt0,kt=0): K-mt0-nq0 and Q-mt0-nq0. V st0/1
        # go first in the filler queue (needed at round 1's AV).
        for g in (
            gen_qk("k", wk_sb, bk_sb, kT_sb, 0, 0),
            gen_qk("q", wq_sb, bq_sb, qT_sb, 0, 0),
        ):
            for _ in g:
                pass
        done.update({("k", 0, 0), ("q", 0, 0)})

        # remaining work ordered by first-use round:
        # K-nq j gates sc(kt=4j); V st gates av(kt=st); block order is
        # (qb0,mt0),(qb0,mt1),(qb1,mt0),... so Q/K-mt1 gate block 1
        fill_q.extend(
            [
                gen_v(0),
                gen_v(1),
                gen_v(2),
                gen_qk("k", wk_sb, bk_sb, kT_sb, 0, 1),
                gen_v(3),
                gen_v(4),
                gen_v(5),
                gen_qk("k", wk_sb, bk_sb, kT_sb, 0, 2),
                gen_v(6),
                gen_v(7),
                gen_v(8),
                gen_qk("k", wk_sb, bk_sb, kT_sb, 0, 3),
                gen_v(9),
                gen_v(10),
                gen_v(11),
                gen_v(12),
                gen_v(13),
                gen_v(14),
                gen_v(15),
                gen_qk("k", wk_sb, bk_sb, kT_sb, 1, 0),
                gen_qk("q", wq_sb, bq_sb, qT_sb, 1, 0),
                gen_qk("k", wk_sb, bk_sb, kT_sb, 1, 1),
                gen_qk("k", wk_sb, bk_sb, kT_sb, 1, 2),
                gen_qk("k", wk_sb, bk_sb, kT_sb, 1, 3),
                gen_qk("q", wq_sb, bq_sb, qT_sb, 0, 1),
                gen_qk("q", wq_sb, bq_sb, qT_sb, 1, 1),
                gen_qk("q", wq_sb, bq_sb, qT_sb, 0, 2),
                gen_qk("q", wq_sb, bq_sb, qT_sb, 1, 2),
                gen_qk("q", wq_sb, bq_sb, qT_sb, 0, 3),
                gen_qk("q", wq_sb, bq_sb, qT_sb, 1, 3),
            ]
        )

        # ---- ACT-paced attention rounds ----
        blocks = [(qb, mt) for qb in range(NQB) for mt in range(2)]
        for bi, (qb, mt) in enumerate(blocks):
            # output projections deferred to the LATE blocks: the filler
            # queue is oversubscribed early (V/K/Q) but drains by block 5,
            # leaving ~200ns/round of PE slack that phaseD fits exactly
            if bi >= 5:
                # block 7 keeps two pieces back: released in the tail so
                # the PE stays busy (and HAM-warm) through the last norm
                # chain's DVE latency
                hold = 2 if bi == 7 else QB // P
                for sti in range(hold):
                    fill_q.append(gen_phaseD(bi - 5, sti))
            hA, hB = 2 * mt, 2 * mt + 1
            q0 = qb * QB
            avA = ps_av.tile([P, 512], F32, tag="avA", name="avA")
            avB = ps_av.tile([P, 512], F32, tag="avB", name="avB")
            pump_until(("q", mt, qb))
            prev = None  # (ex, kt) awaiting its AV matmuls
            for kt in range(ST):
                pump_until(("k", mt, kt // 4))
                sc = ps_sc.tile([P, 1024], F32, tag="sc")
                # the two heads' score matmuls row-tile (rows 0-63 / 64-127
                # of the PE array) and run concurrently
                nc.tensor.matmul(
                    sc[:, 0:512],
                    kT_sb[0:64, mt, kt * P : (kt + 1) * P],
                    qT_sb[0:64, mt, q0 : q0 + QB],
                    start=True,
                    stop=True,
                )
                nc.tensor.matmul(
                    sc[:, 512:1024],
                    kT_sb[64:128, mt, kt * P : (kt + 1) * P],
                    qT_sb[64:128, mt, q0 : q0 + QB],
                    start=True,
                    stop=True,
                )
                ex = expool.tile([P, 1024], BF16, tag="ex")
                nc.scalar.activation(
                    ex[:],
                    sc[:],
                    mybir.ActivationFunctionType.Exp,
                    scale=0.125,
                )
                if prev is not None:
                    pex, pkt = prev
                    pump_until(("v", pkt))
                    nc.tensor.matmul(
                        avA[0:65, :],
                        v_sb[:, pkt, hA * 65 : hA * 65 + 65],
                        pex[:, 0:512],
                        start=(pkt == 0),
                        stop=(pkt == ST - 1),
                    )
                    nc.tensor.matmul(
                        avB[0:65, :],
                        v_sb[:, pkt, hB * 65 : hB * 65 + 65],
                        pex[:, 512:1024],
                        start=(pkt == 0),
                        stop=(pkt == ST - 1),
                    )
                prev = (ex, kt)
                pump(2 if bi == 0 else 1)
            pex, pkt = prev
            pump_until(("v", pkt))
            nc.tensor.matmul(
                avA[0:65, :],
                v_sb[:, pkt, hA * 65 : hA * 65 + 65],
                pex[:, 0:512],
                start=False,
                stop=True,
            )
            nc.tensor.matmul(
                avB[0:65, :],
                v_sb[:, pkt, hB * 65 : hB * 65 + 65],
                pex[:, 512:1024],
                start=False,
                stop=True,
            )

            # evict av -> SBUF (frees the av banks), 1/l on DVE, then the
            # normalize runs as filler during the next block. In the last
            # block, head B's copies go to the (now idle) ACT engine so
            # the two chains overlap instead of serializing on DVE.
            last = bi == len(blocks) - 1
            for h, av in ((hA, avA), (hB, avB)):
                on_act = last and h == hB
                cp = nc.scalar.copy if on_act else nc.vector.tensor_copy
                ue = npool.tile([65, 512], F32, tag="ue")
                cp(ue[:], av[0:65, :])
                # reciprocal_approx_fast misbehaves at partition offset 64:
                # shift l down to partition 0 first
                l0 = npool.tile([1, 512], F32, tag="l0")
                cp(l0[:], ue[64:65, :])
                rinv = npool.tile([1, 512], F32, tag="rinv")
                nc.vector.reciprocal_approx_fast(rinv[:], l0[:])
                fill_q.append(gen_norm(qb, h, ue, rinv))

        # tail: the held-back phaseD(2) pieces land after the last norms
        # in the queue, giving the PE work during the norm chain; then the
        # last q-block's projection
        fill_q.append(gen_phaseD(2, 2))
        fill_q.append(gen_phaseD(2, 3))
        pump(1 << 20)
        for sti in range(QB // P):
            phaseD_tail(NQB - 1, sti)


_NC_CACHE = None


def get_nc():
    global _NC_CACHE
    if _NC_CACHE is None:
        _NC_CACHE = build_nc()
    return _NC_CACHE


def make_in_maps(x, Wq, bq, Wk, bk, Wv, bv, Wo, bo):
    import ml_dtypes

    bf16 = ml_dtypes.bfloat16
    in_maps = []
    for c in range(8):
        b, g = c // 4, c % 4
        sl = slice(g * DG, (g + 1) * DG)
        in_maps.append(
            {
                "xT": np.ascontiguousarray(x[b].T.astype(bf16)),
                "wq": np.ascontiguousarray(Wq[:, sl].astype(bf16)),
                "wk": np.ascontiguousarray(Wk[:, sl].astype(bf16)),
                "wv": np.ascontiguousarray(Wv[:, sl].astype(bf16)),
                "wo": np.ascontiguousarray(Wo[sl, :].astype(bf16)),
                "bq": np.ascontiguousarray(bq[sl].reshape(2, P).T),
                "bk": np.ascontiguousarray(bk[sl].reshape(2, P).T),
                "bv": np.ascontiguousarray(bv[sl].reshape(1, DG).astype(bf16)),
            }
        )
    return in_maps


def kernel(x, Wq, bq, Wk, bk, Wv, bv, Wo, bo, _run_kwargs=None):
    from concourse.bass_utils import run_bass_kernel_spmd

    x = np.asarray(x, dtype=np.float32)
    nc = get_nc()
    in_maps = make_in_maps(
        x,
        np.asarray(Wq, np.float32),
        np.asarray(bq, np.float32),
        np.asarray(Wk, np.float32),
        np.asarray(bk, np.float32),
        np.asarray(Wv, np.float32),
        np.asarray(bv, np.float32),
        np.asarray(Wo, np.float32),
        np.asarray(bo, np.float32),
    )
    res = run_bass_kernel_spmd(
        nc, in_maps, core_ids=list(range(8)), **(_run_kwargs or {})
    )
    bo = np.asarray(bo, np.float32)
    outp = np.empty((2, S, D), dtype=np.float32)
    for b in range(2):
        acc = res.results[4 * b]["out"].astype(np.float32)
        for g in range(1, 4):
            acc = acc + res.results[4 * b + g]["out"].astype(np.float32)
        outp[b] = acc + bo[None, :]
    kernel.last_result = res
    return outp
